# revision 1
# baseline (speedup 1.0000x reference)
"""NT-Xent (SimCLR) loss on 8 Trainium2 NeuronCores.

Full inputs z1, z2: [4096, 256] f32.  z = concat -> [8192, 256], rows
L2-normalized, sim = zn @ zn.T / 0.5 with the diagonal masked out, row
log-softmax, loss = -mean over rows of logp[i, pair(i)].

Sharding: data-parallel over rows.  Every core receives the full z1/z2
(to build the all-column normalized z^T it needs as the matmul moving
operand) plus its own 1024-row block (zrows) and the paired block
(zpair).  The host only slices inputs and averages the 8192 row losses.

Device algorithm (per core):
  - denom_i = sum_j exp(2*cos_ij - 2) - exp(2*selfdot_i - 2); constant
    shift works because sim_ii = 2.0 is always the row max.
  - Row blocks stay un-normalized; the row's 2/||z_i|| is folded into
    the exp's per-partition scale operand.
  - exp + row-sum happen in one ScalarE activation per row-block over
    [128, 2048] PSUM tiles (accum_out); ScalarE runs ONLY Exp in steady
    state, so its 64us of exp work is the roofline.
  - square+reduce are FUSED per z-tile via scalar_tensor_tensor accum
    on DVE; rsqrt on DVE (bit-trick + Newton); normalize on GPSIMD
    (groups 1-3) writing bf16.
  - ALL transposes ride the DMA XBAR (dma_start_transpose, 14ns per
    16x128 tile): one instruction per half-group turns the normalized
    [128 rows, 8 tiles x 256 d] slab into the [128 d, tile, k, col]
    moving operand.  No PE transposes, no PSUM contention, no
    PSUM->SBUF copies.
  - loss_row = 2 + log(denom_i - diag_i) - 2*pos_i.

Schedule: software-pipelined across the 4 column groups (group g+1's
stats chain runs under group g's matmul/exp phase).  Group 0 is split
into quarters (512-col exp passes for q0/q1) to shorten startup; a few
identity transposes warm the PE p-state before the first real matmul.
"""

import numpy as np
from contextlib import ExitStack

import concourse.bass as bass
import concourse.bacc as bacc
import concourse.mybir as mybir
import concourse.tile as tile
from concourse import masks
from concourse.bass import ts
from concourse.bass_utils import run_bass_kernel_spmd

F32 = mybir.dt.float32
I32 = mybir.dt.int32
AF = mybir.ActivationFunctionType
ALU = mybir.AluOpType

P = 128          # partitions
D = 256          # feature dim
N = 4096         # rows per z1 / z2
R = 2 * N        # 8192 total rows
NCORES = 8
RPC = R // NCORES          # 1024 rows per core
NB = RPC // P              # 8 row blocks per core
NT = R // P                # 64 natural tiles of the full z
GROUPS = 4                 # process full z in 4 groups of 16 tiles
TPG = NT // GROUPS         # 16 tiles per group = 2048 sim columns
HT = TPG // 2              # 8 tiles per half-group
QT = TPG // 4              # 4 tiles per quarter-group
SCALE = 2.0                # 1/temperature
BF16 = mybir.dt.bfloat16
MM_DT = BF16               # matmul operand dtype
NDEN = 6                   # denom accum slots (g0: 0-2; g1-3: one each)


def _dve_rsqrt(nc, scratch, r_view, a_view, magic_view, n, tag, steps=1):
    """r = 1/sqrt(a) entirely on DVE: int bit-trick seed + Newton steps."""
    ri = r_view.bitcast(I32)
    ai = a_view.bitcast(I32)
    nc.vector.tensor_scalar(
        out=ri, in0=ai, scalar1=1, scalar2=None, op0=ALU.arith_shift_right
    )
    nc.vector.tensor_tensor(out=ri, in0=magic_view, in1=ri, op=ALU.subtract)
    for s in range(steps):
        t1 = scratch.tile([P, n], F32, tag=tag, bufs=4, name=f"{tag}_n{s}")
        nc.vector.tensor_tensor(out=t1[:], in0=r_view, in1=r_view, op=ALU.mult)
        nc.vector.tensor_tensor(out=t1[:], in0=t1[:], in1=a_view, op=ALU.mult)
        nc.vector.tensor_scalar(
            out=t1[:], in0=t1[:], scalar1=-0.5, scalar2=1.5,
            op0=ALU.mult, op1=ALU.add,
        )
        nc.vector.tensor_tensor(out=r_view, in0=r_view, in1=t1[:], op=ALU.mult)


def build_nc(loop_n=None, stage="full"):
    nc = bacc.Bacc(None, target_bir_lowering=False, debug=False)

    z1 = nc.declare_dram_parameter("z1", [N, D], F32, isOutput=False)
    z2 = nc.declare_dram_parameter("z2", [N, D], F32, isOutput=False)
    zrows = nc.declare_dram_parameter("zrows", [RPC, D], BF16, isOutput=False)
    zrowsT = nc.declare_dram_parameter("zrowsT", [P, NB, 2, P], BF16,
                                       isOutput=False)
    zpair = nc.declare_dram_parameter("zpair", [RPC, D], BF16, isOutput=False)
    out = nc.declare_dram_parameter("loss_rows", [P, NB], F32, isOutput=True)

    with tile.TileContext(nc) as tc, ExitStack() as ctx:
        consts = ctx.enter_context(tc.tile_pool(name="consts", bufs=1))
        small = ctx.enter_context(tc.tile_pool(name="small", bufs=1))
        scratch = ctx.enter_context(tc.tile_pool(name="scratch", bufs=2))
        zgp = ctx.enter_context(tc.tile_pool(name="zgp", bufs=2))
        znt_pool = ctx.enter_context(tc.tile_pool(name="znt", bufs=1))
        zr_pool = ctx.enter_context(tc.tile_pool(name="zrp", bufs=1))
        psum = ctx.enter_context(
            tc.tile_pool(name="psum", bufs=2, space=bass.MemorySpace.PSUM)
        )
        expout = ctx.enter_context(tc.tile_pool(name="expout", bufs=2))

        identity = consts.tile([P, P], F32)
        masks.make_identity(nc, identity[:])
        negtwo = consts.tile([P, 1], F32)
        nc.gpsimd.memset(negtwo[:], -2.0)
        magic = consts.tile([P, TPG], I32)
        nc.gpsimd.memset(magic[:], 0x5F3759DF)
        # pull the exp/ln table load to t=0 so it never delays the first
        # real exp
        tiny = consts.tile([P, 1], F32)
        nc.scalar.activation(tiny[:], negtwo[:], AF.Exp)

        loop_cm = tc.For_i(0, loop_n, 1) if loop_n else ExitStack()
        ctx.enter_context(loop_cm)

        # ---------- prologue DMAs (queue order = land order) --------------
        zg = [None] * GROUPS
        zg[0] = zgp.tile([P, TPG, D], F32, tag="zg", bufs=3, name="zg0")
        nc.sync.dma_start(
            zg[0][:, 0:QT, :],
            z1[0 : QT * P, :].rearrange("(p r) d -> p r d", r=QT),
        )
        zrawT = zr_pool.tile([P, NB, 2, P], MM_DT, tag="zrawT", bufs=2)
        nc.sync.dma_start(zrawT[:], zrowsT[:, :, :, :])
        zr = zr_pool.tile([P, NB, D], BF16, tag="zr", bufs=2)
        nc.sync.dma_start(zr[:], zrows.rearrange("(p r) d -> p r d", r=NB))
        for q in range(1, 4):
            nc.sync.dma_start(
                zg[0][:, q * QT : (q + 1) * QT, :],
                z1[q * QT * P : (q + 1) * QT * P, :].rearrange(
                    "(p r) d -> p r d", r=QT),
            )
        def dma_group_half(g, h):
            src = z1 if g < GROUPS // 2 else z2
            row0 = (g % (GROUPS // 2)) * (TPG * P) + h * (HT * P)
            if zg[g] is None:
                zg[g] = zgp.tile([P, TPG, D], F32, tag="zg", bufs=3,
                                 name=f"zg{g}")
            nc.sync.dma_start(
                zg[g][:, h * HT : (h + 1) * HT, :],
                src[row0 : row0 + HT * P, :].rearrange("(p r) d -> p r d",
                                                       r=HT),
            )

        # ---------- small state ------------------------------------------
        sa = small.tile([P, 2 * NB], F32, tag="sa", bufs=2)
        rn2 = small.tile([P, 2 * NB], F32, tag="rn2", bufs=2)
        rnr2 = small.tile([P, NB], F32, tag="rnr2", bufs=2)
        rawpos = small.tile([P, NB], F32, tag="rawpos", bufs=2)
        denoms = small.tile([P, NB, NDEN], F32, tag="den", bufs=2)
        # group-0's coarse-grain blocks only write slot 0
        nc.gpsimd.memset(denoms[:], 0.0)

        def sqsum(dst_col_view, src_view, n_tiles, tag):
            """fused square+reduce: per z-tile scalar_tensor_tensor with
            accum_out (one [P,1] sum per tile)."""
            for t in range(n_tiles):
                dummy = scratch.tile([P, D], F32, tag="stt", bufs=2,
                                     name=f"stt{tag}_{t}")
                nc.vector.scalar_tensor_tensor(
                    out=dummy[:],
                    in0=src_view[:, t, :], scalar=1.0, in1=src_view[:, t, :],
                    op0=ALU.mult, op1=ALU.mult,
                    accum_out=dst_col_view[:, t : t + 1],
                )

        # moving operand tiles [128, tiles, k, 128] bf16
        znth = [[None, None] for _ in range(GROUPS)]
        zgn = {}

        def stats_batch(g, t_lo, nst, tag, norm_on_pool=True):
            """fused sq+reduce, rsqrt, normalize (bf16 out) for zg[g] tiles
            [t_lo, t_lo+nst).  Per-batch stat tiles: sharing one sqf/rnf
            tensor would chain every batch behind the previous one through
            tile-granularity WAR hazards."""
            tis = slice(t_lo, t_lo + nst)
            sq_b = scratch.tile([P, nst], F32, tag="sqb", bufs=4,
                                name=f"sqb{tag}")
            rn_b = scratch.tile([P, nst], F32, tag="rnb", bufs=4,
                                name=f"rnb{tag}")
            sqsum(sq_b[:], zg[g][:, tis, :], nst, tag)
            nc.vector.tensor_scalar_max(sq_b[:], sq_b[:], 1e-16)
            _dve_rsqrt(nc, scratch, rn_b[:], sq_b[:], magic[:, 0:nst],
                       nst, "nwt_g")
            # zgn is k-major [P, 2, nst, 128] so each k-plane is contiguous:
            # the DMA-XBAR transpose needs a 2D input and the matmul moving
            # operand a 2D AP per k
            zn = scratch.tile([P, 2, nst, P], BF16, tag="zgn", bufs=4,
                              name=f"zgn{tag}")
            zgn[(g, t_lo)] = zn
            eng = nc.gpsimd if norm_on_pool else nc.vector
            # normalize in quarter chunks so each transpose can start as
            # soon as its chunk is done
            step = min(nst, QT)
            for s0 in range(0, nst, step):
                for k in range(2):
                    eng.tensor_tensor(
                        out=zn[:, k, s0 : s0 + step, :],
                        in0=zg[g][:, t_lo + s0 : t_lo + s0 + step,
                                 k * P : (k + 1) * P],
                        in1=rn_b[:, s0 : s0 + step].to_broadcast((P, step, P)),
                        op=ALU.mult,
                    )

        def transp_dma(g, t_lo, nst, dst, sub_lo=0):
            """per-k DMA-XBAR transposes: zgn (g, t_lo) tiles [sub_lo,
            sub_lo+nst) -> dst [128, 2, nst, 128] (k-major)."""
            zn = zgn[(g, t_lo)]
            for k in range(2):
                nc.sync.dma_start_transpose(
                    dst[:, k, :, :],
                    zn[:, k, sub_lo : sub_lo + nst, :].rearrange(
                        "p j c -> p (j c)"),
                )

        def transp_batch(g, h, t_lo, nst):
            """transpose in quarter chunks (pipelines with the chunked
            normalize)."""
            if znth[g][h] is None:
                znth[g][h] = znt_pool.tile(
                    [P, 2, HT, P], MM_DT, tag=f"znt{h}", bufs=3,
                    name=f"znt{g}_{h}",
                )
            j0 = t_lo - h * HT
            transp_dma(g, t_lo, nst, znth[g][h][:, :, j0 : j0 + nst, :])

        # ---------- group-0 stats at quarter granularity ------------------
        # q0's chain first (it gates the first matmul), then zr's stats
        # (only needed by the first exp), then q1-q3
        znq = []

        def g0_quarter(q):
            stats_batch(0, q * QT, QT, tag=f"g0q{q}")
            znq.append(znt_pool.tile([P, 2, QT, P], MM_DT, tag=f"znq{q}",
                                     bufs=2, name=f"znq{q}"))
            transp_dma(0, q * QT, QT, znq[q][:])

        g0_quarter(0)

        # zr stats: fused sq+reduce then rsqrt; rnr2 = 2/|z| feeds exp scale
        sqsum(sa[:, 0:NB], zr[:], NB, "zr")
        nc.vector.tensor_scalar_max(sa[:, 0:NB], sa[:, 0:NB], 1e-16)
        _dve_rsqrt(nc, scratch, rn2[:, 0:NB], sa[:, 0:NB], magic[:, 0:NB],
                   NB, "nwt_r")
        nc.vector.tensor_scalar_mul(rnr2[:], rn2[:, 0:NB], SCALE)

        for q in range(1, 4):
            g0_quarter(q)

        # deferred bulk DMAs (group-1 halves first: needed mid-group-0)
        dma_group_half(1, 0)
        dma_group_half(1, 1)
        zp = zr_pool.tile([P, NB, D], BF16, tag="zp", bufs=2)
        nc.sync.dma_start(zp[:], zpair.rearrange("(p r) d -> p r d", r=NB))

        # PE p-state warmup: junk transposes of the identity while the
        # pipeline fills
        junk = psum.tile([P, P], F32, tag="ps", name="junk")
        for w in range(10):
            nc.tensor.transpose(junk[:], identity[:], identity[:])

        # ---------- main pipelined loop over groups ----------------------
        def mm_block(o, b, rhs_tile, j0, nj):
            """accumulating matmuls over both k halves in 512-col chunks
            (ISA caps the moving operand's contiguous run at 512)"""
            for k in range(2):
                for c0 in range(0, nj * P, 512):
                    nc.tensor.matmul(
                        o[:, c0 : c0 + 512],
                        zrawT[:, b, k, :],
                        rhs_tile[:, k, j0 + c0 // P : j0 + (c0 + 512) // P, :]
                        .rearrange("p j c -> p (j c)"),
                        start=(k == 0),
                        stop=(k == 1),
                    )

        for g in range(GROUPS):
            if g == 0:
                # blocks b0-b3: fine-grain passes (q0, q1, then q2+q3) keep
                # ScalarE busy while the later quarter chains finish; blocks
                # b4-b7 run one full 2048-col exp each (all quarters ready
                # by then), saving per-instruction overhead
                for qq in range(2):
                    for b in range(4):
                        pm = psum.tile([P, 1, 512], F32, tag="ps",
                                       name=f"pm0_{b}_q{qq}")
                        mm_block(pm[:, 0, :], b, znq[qq][:], 0, QT)
                        eo = expout.tile([P, 1, 512], MM_DT, tag="eo",
                                         bufs=2, name=f"eo0_{b}_q{qq}")
                        nc.scalar.activation(
                            eo[:], pm[:], AF.Exp,
                            bias=negtwo[:], scale=rnr2[:, b : b + 1],
                            accum_out=denoms[:, b, qq : qq + 1],
                        )
                        if qq == 0 and b == 0:
                            stats_batch(1, 0, HT, "g1h0")
                            transp_batch(1, 0, 0, HT)
                        elif qq == 1 and b == 0:
                            stats_batch(1, HT, HT, "g1h1")
                            transp_batch(1, 1, HT, HT)
                        elif qq == 1 and b == 3:
                            dma_group_half(2, 0)
                for b in range(4):
                    pm = psum.tile([P, 2, 512], F32, tag="ps",
                                   name=f"pm0_{b}_h1")
                    mm_block(pm[:, 0, :], b, znq[2][:], 0, QT)
                    mm_block(pm[:, 1, :], b, znq[3][:], 0, QT)
                    eo = expout.tile([P, 2, 512], MM_DT, tag="eo",
                                     bufs=2, name=f"eo0_{b}_h1")
                    nc.scalar.activation(
                        eo[:], pm[:], AF.Exp,
                        bias=negtwo[:], scale=rnr2[:, b : b + 1],
                        accum_out=denoms[:, b, 2:3],
                    )
                    if b == 3:
                        dma_group_half(2, 1)
                for b in range(4, NB):
                    pm = psum.tile([P, 4, 512], F32, tag="ps",
                                   name=f"pm0_{b}_full")
                    o = pm[:].rearrange("p q c -> p (q c)")
                    for qq in range(4):
                        mm_block(o[:, qq * 512 : (qq + 1) * 512], b,
                                 znq[qq][:], 0, QT)
                    eo = expout.tile([P, 4, 512], MM_DT, tag="eo",
                                     bufs=2, name=f"eo0_{b}_full")
                    nc.scalar.activation(
                        eo[:], pm[:], AF.Exp,
                        bias=negtwo[:], scale=rnr2[:, b : b + 1],
                        accum_out=denoms[:, b, 0:1],
                    )
            else:
                dslot = g + 2
                for b in range(NB):
                    if False:
                        # split exp per half (own psum tiles): phase start
                        # only waits on the h0 transpose, not h1's
                        for h in range(2):
                            pm = psum.tile([P, 2, 512], F32, tag="ps",
                                           name=f"pm{g}_{b}_h{h}")
                            mm_block(pm[:].rearrange("p q c -> p (q c)"),
                                     b, znth[g][h][:], 0, HT)
                            eo = expout.tile([P, 2, 512], MM_DT, tag="eo",
                                             bufs=2, name=f"eo{g}_{b}_h{h}")
                            nc.scalar.activation(
                                eo[:], pm[:], AF.Exp,
                                bias=negtwo[:], scale=rnr2[:, b : b + 1],
                                accum_out=denoms[:, b, dslot + h : dslot + h + 1],
                            )
                    else:
                        pm = psum.tile([P, 4, 512], F32, tag="ps",
                                       name=f"pm{g}_{b}")
                        o = pm[:].rearrange("p q c -> p (q c)")
                        mm_block(o[:, 0:1024], b, znth[g][0][:], 0, HT)
                        mm_block(o[:, 1024:2048], b, znth[g][1][:], 0, HT)
                        eo = expout.tile([P, 4, 512], MM_DT, tag="eo",
                                         bufs=2, name=f"eo{g}_{b}")
                        nc.scalar.activation(
                            eo[:], pm[:], AF.Exp,
                            bias=negtwo[:], scale=rnr2[:, b : b + 1],
                            accum_out=denoms[:, b, dslot : dslot + 1],
                        )
                    if g + 1 < GROUPS:
                        if b == 0:
                            stats_batch(g + 1, 0, HT, f"g{g+1}h0")
                            transp_batch(g + 1, 0, 0, HT)
                        elif b == 2 and g + 2 < GROUPS:
                            dma_group_half(g + 2, 0)
                        elif b == 4:
                            stats_batch(g + 1, HT, HT, f"g{g+1}h1")
                            transp_batch(g + 1, 1, HT, HT)
                        elif b == 6 and g + 2 < GROUPS:
                            dma_group_half(g + 2, 1)
                    if g == 2 and b == 1:
                        # epilogue-only work, placed in late-pipeline slack:
                        # zp stats + positive-pair dot products
                        sqsum(sa[:, NB:], zp[:], NB, "zp")
                        nc.vector.tensor_scalar_max(sa[:, NB:], sa[:, NB:],
                                                    1e-16)
                        _dve_rsqrt(nc, scratch, rn2[:, NB:], sa[:, NB:],
                                   magic[:, 0:NB], NB, "nwt_p")
                        posm = scratch.tile([P, NB, D], F32, tag="posm",
                                            bufs=1, name="posm")
                        nc.gpsimd.tensor_tensor(out=posm[:], in0=zr[:],
                                                in1=zp[:], op=ALU.mult)
                        nc.vector.tensor_reduce(
                            rawpos[:], posm[:], axis=mybir.AxisListType.X,
                            op=ALU.add
                        )

        # ---------- epilogue: per-row loss --------------------------------
        sd = small.tile([P, NB], F32)
        nc.vector.tensor_tensor(out=sd[:], in0=sa[:, 0:NB], in1=rn2[:, 0:NB],
                                op=ALU.mult)
        nc.vector.tensor_tensor(out=sd[:], in0=sd[:], in1=rn2[:, 0:NB],
                                op=ALU.mult)
        diag = small.tile([P, NB], F32)
        nc.scalar.activation(diag[:], sd[:], AF.Exp, bias=negtwo[:],
                             scale=SCALE)
        posx = small.tile([P, NB], F32)
        nc.vector.tensor_tensor(out=posx[:], in0=rawpos[:], in1=rn2[:, 0:NB],
                                op=ALU.mult)
        nc.vector.tensor_tensor(out=posx[:], in0=posx[:], in1=rn2[:, NB:],
                                op=ALU.mult)

        denom = small.tile([P, NB], F32)
        nc.vector.tensor_reduce(
            denom[:], denoms[:], axis=mybir.AxisListType.X, op=ALU.add
        )
        nc.vector.tensor_tensor(out=denom[:], in0=denom[:], in1=diag[:],
                                op=ALU.subtract)
        logd = small.tile([P, NB], F32)
        nc.scalar.activation(logd[:], denom[:], AF.Ln)
        loss = small.tile([P, NB], F32, tag="loss", bufs=2)
        nc.vector.tensor_scalar_mul(loss[:], posx[:], -2.0)
        nc.vector.tensor_tensor(out=loss[:], in0=loss[:], in1=logd[:],
                                op=ALU.add)
        nc.vector.tensor_scalar_add(loss[:], loss[:], 2.0)

        # host mean is order-invariant: DMA the [128, 8] loss tile
        # directly (no PE transpose, no PSUM slot blocking the next
        # iteration's matmul tiles)
        nc.sync.dma_start(out[:, :], loss[:])

    nc.compile()
    return nc


_NC = None


def _get_nc():
    global _NC
    if _NC is None:
        _NC = build_nc()
    return _NC


def _in_maps(z1, z2):
    import ml_dtypes

    bf16 = ml_dtypes.bfloat16
    z1 = np.ascontiguousarray(z1, dtype=np.float32)
    z2 = np.ascontiguousarray(z2, dtype=np.float32)
    z = np.concatenate([z1, z2], axis=0)
    zb = z.astype(bf16)
    maps = []
    for c in range(NCORES):
        lo = c * RPC
        plo = (lo + N) % R
        zrows = np.ascontiguousarray(zb[lo : lo + RPC])
        # [P, b, k, c] with zrowsT[p, b, k, c] = zrows[c*NB + b, k*128 + p]
        zrowsT = np.ascontiguousarray(
            zrows.reshape(P, NB, 2, P).transpose(3, 1, 2, 0)
        )
        maps.append(
            {
                "z1": z1,
                "z2": z2,
                "zrows": zrows,
                "zrowsT": zrowsT,
                "zpair": np.ascontiguousarray(zb[plo : plo + RPC]),
            }
        )
    return maps


def run(z1, z2, trace=False, **kwargs):
    nc = _get_nc()
    res = run_bass_kernel_spmd(
        nc, _in_maps(z1, z2), list(range(NCORES)), trace=trace, **kwargs
    )
    rows = np.concatenate(
        [np.asarray(res.results[c]["loss_rows"]).reshape(-1) for c in range(NCORES)]
    )
    return np.float32(rows.mean()), res


def kernel(z1, z2):
    loss, _ = run(z1, z2)
    return loss

